# revision 21
# baseline (speedup 1.0000x reference)
"""CovariantEvolutionBlock Trainium2 kernel (v2).

Strategy: token-parallel over B*L across 8 cores (512 tokens/core), zero
collectives. Each core recomputes full-batch K/V for attention (inputs are
rotated per-core so "own" tokens are always columns 0:512; sigmoid attention
is permutation-invariant over keys). Activations are kept feature-major
[dims, tokens] on-chip so matmul chains need no transposes.

All dense linears except cu*/m* run in fp8(e4m3) with DoubleRow perf mode;
weights are host-scaled by 256 to escape the e4m3 subnormal range and
descaled inside the PSUM evictions. cu/m stay bf16 (fp8 there costs ~2.4e-2
rel err, over the 2e-2 gate).

v2 scheduling changes vs v1:
- DMAs spread across engine queues (weights/biases on scalar, vw + z32/c32
  prefetch on gpsimd, activations+slabs on sync); big memsets on gpsimd.
- rms square-sums run as fp8 DoubleRow matmuls against a [128,2,1] ones
  stationary (half the passes), with x^2 computed fp8*fp8 on vector.
- g1 interleaved into the attention loop (tensor has slack there: the phase
  is scalar-sigmoid-bound), g2+o right after the head loop; phase4 is
  cu1 -> m1 -> cu2 -> m2 with cu1's contraction reordered (conn, dz first,
  z1 last) so each linear starts before its predecessor's tail lands.
- weight slabs prefetch 2-deep from a pool opened before attention.
- raw z/c staged per-chunk to bf16 (halves their SBUF) -- z1/conn_new pick
  up ~0.3% extra rel err from the bf16 carry, well within budget.
"""

import sys

try:
    import concourse.bass as bass  # noqa: F401
except ImportError:
    sys.path.insert(0, "/opt/trn_rl_repo")

import numpy as np
import ml_dtypes

import concourse.bacc as bacc
import concourse.tile as tile
import concourse.mybir as mybir
from concourse.bass_utils import run_bass_kernel_spmd

F32 = mybir.dt.float32
BF16 = mybir.dt.bfloat16
FP8 = mybir.dt.float8e4
AF = mybir.ActivationFunctionType
ALU = mybir.AluOpType
DR = mybir.MatmulPerfMode.DoubleRow

B, L, D, H, HD = 2, 2048, 1024, 16, 64
EPS = 1e-6
NCORES = 8
TOK = 512          # own tokens per core
KEYS = 2048        # keys per batch
KC = D // 128      # 8 feature chunks of 128
NTB = KEYS // TOK  # 4 token blocks per batch
OBW = 256          # out-block width (2 m-chunks) per psum tile
WSCALE = 256.0     # fp8 weight scale (weights ~N(0,0.02) are subnormal raw)
DS = 1.0 / WSCALE

# name -> (n_in, n_out) for ob-major packed linears
LINS = {
    "fw1": (D, 2 * D), "fw2": (2 * D, D),
    "gw1": (2 * D, D), "gw2": (D, D),
    "qw": (2 * D, D), "kw": (2 * D, D),
    "ow": (D, D),
    "cuw1": (3 * D, 2 * D), "cuw2": (2 * D, D),
    "mw1": (D, 4 * D), "mw2": (4 * D, D),
}
# cu/m weight+act quantization lands unattenuated on the outputs
# (conn_new = c + cu2(...), z2 = z1 + m2(...)): fp8 there measures ~2.4e-2
# end-to-end, over the 2e-2 gate. Everything else is attenuated (dt=0.1 on
# dz, 1/rel_sum on ctx) -> fp8 ok.
FP8_LINS = {"fw1", "fw2", "gw1", "gw2", "qw", "kw", "ow", "cuw1", "mw1"}


def _bias_ap(dram_ap):
    # [dim] -> [128, dim//128]: tile[p, c] = bias[c*128 + p]
    return dram_ap.rearrange("(c p) -> p c", p=128)


def build_program(dt_val: float, temp_val: float):
    nc = bacc.Bacc("TRN2", target_bir_lowering=False, debug=False,
                   num_devices=NCORES)

    d_in = {}
    for name, shape, dt in [
        ("zT", [D, KEYS], F32), ("cT", [D, KEYS], F32),
        ("vwK", [128, KC, D], FP8),  # k-major: streamed operand
        ("fb1", [128, 16], F32), ("fb2", [128, 8], F32),
        ("gb1", [128, 8], F32), ("gb2", [128, 8], F32),
        ("cub1", [128, 16], F32), ("cub2", [128, 8], F32),
        ("mb1", [128, 32], F32), ("mb2", [128, 8], F32),
        ("wz", [128, 8], F32), ("wc", [128, 8], F32),
        ("wmlp", [128, 8], F32),
    ]:
        d_in[name] = nc.dram_tensor(name, shape, dt, kind="ExternalInput").ap()
    for name, (n_in, n_out) in LINS.items():
        wdt = FP8 if name in FP8_LINS else BF16
        d_in[name + "P"] = nc.dram_tensor(
            name + "P", [128, n_out // OBW, n_in // 128, OBW], wdt,
            kind="ExternalInput").ap()

    z2T_d = nc.dram_tensor("z2T", [D, TOK], F32, kind="ExternalOutput").ap()
    connT_d = nc.dram_tensor("connT", [D, TOK], F32, kind="ExternalOutput").ap()

    sig_scale = float(temp_val) * (HD ** -0.5)

    with tile.TileContext(nc) as tc:
        _emit(nc, tc, d_in, z2T_d, connT_d, float(dt_val), sig_scale)
    nc.compile()
    return nc


def _emit(nc, tc, d_in, z2T_d, connT_d, dt_val, sig_scale):
    from contextlib import ExitStack

    ctx = ExitStack()
    with ctx:
        # ---------- persistent pools ----------
        const = ctx.enter_context(tc.tile_pool(name="const", bufs=1))
        persist = ctx.enter_context(tc.tile_pool(name="persist", bufs=1))

        # rms weights + ones/eps needed immediately; weight/bias DMAs ride
        # the scalar queue so sync is free for the z/c activation chunks.
        # The V-projection weight goes first: V's matmuls need it ~12us in.
        vw_pool = ctx.enter_context(tc.tile_pool(name="vwp", bufs=1))
        vw = vw_pool.tile([128, KC, D], FP8, tag="vw")
        nc.scalar.dma_start(out=vw[:], in_=d_in["vwK"][:, :, :])
        wcol = {}
        for name in ["wz", "wc", "wmlp"]:
            t = const.tile([128, KC], F32, tag=name)
            nc.scalar.dma_start(out=t[:], in_=d_in[name][:, :])
            wcol[name] = t
        ones_col = const.tile([128, 1], BF16, tag="ones")
        nc.vector.memset(ones_col[:], 1.0)
        # [128,2,16] so the DR chunk-pair stride is 16B-aligned; used as
        # a [128,2,1] slice
        ones2 = const.tile([128, 2, 16], FP8, tag="ones2")
        nc.vector.memset(ones2[:], 1.0)
        eps1 = const.tile([1, 1], F32, tag="eps1")
        nc.vector.memset(eps1[:], EPS)
        warm = const.tile([1, 2], F32, tag="warm")
        nc.scalar.activation(warm[0:1, 0:1], eps1[:], AF.Square)
        nc.scalar.activation(warm[0:1, 1:2], eps1[:], AF.Sqrt)

        # persistent activations (own tokens, feature-major, fp8)
        cn_own = persist.tile([128, KC, TOK], FP8, tag="cn_own")
        attnT = persist.tile([128, KC, TOK], FP8, tag="attnT")
        mlp = ctx.enter_context(tc.tile_pool(name="mlp", bufs=1))
        mlp2 = ctx.enter_context(tc.tile_pool(name="mlp2", bufs=1))
        outp = ctx.enter_context(tc.tile_pool(name="outp", bufs=1))

        bias = {}

        def load_biases(names):
            for name in names:
                n = d_in[name].shape[1]
                t = const.tile([128, n], F32, tag=name)
                nc.scalar.dma_start(out=t[:], in_=d_in[name][:, :])
                bias[name] = t

        # ---------- generic feature-major linear ----------
        # Weights stream as one packed slab per 256-wide out-block.
        # fp8 linears use DoubleRow (rhs_fn returns [128,2,TOK] pairs);
        # bf16 linears use plain matmul (rhs_fn returns [128,TOK] chunks).
        def _emit_ob(wname, ob, rhs_fn, evict_fn, wpool, pspool, wk,
                     korder=None):
            n_in, n_out = LINS[wname]
            fp8 = wname in FP8_LINS
            wdt = FP8 if fp8 else BF16
            kcn = n_in // 128
            ksteps = kcn // 2 if fp8 else kcn
            ks = list(range(ksteps)) if korder is None else korder
            wP = d_in[wname + "P"]
            w = wpool.tile([128, wk, OBW], wdt, tag="wslab")
            nc.sync.dma_start(out=w[:, 0:kcn, :], in_=wP[:, ob, :, :])
            ps = pspool.tile([128, 2, 512], F32, tag="lin")
            for i, k in enumerate(ks):
                for m in range(2):
                    if fp8:
                        nc.tensor.matmul(
                            ps[:, m, :TOK],
                            w[:, 2 * k:2 * k + 2, m * 128:(m + 1) * 128],
                            rhs_fn(k), start=(i == 0),
                            stop=(i == ksteps - 1), perf_mode=DR)
                    else:
                        nc.tensor.matmul(
                            ps[:, m, :TOK],
                            w[:, k, m * 128:(m + 1) * 128],
                            rhs_fn(k), start=(i == 0),
                            stop=(i == ksteps - 1))
            evict_fn(ob, ps[:, :, :TOK])

        def linear_fm(wname, rhs_fn, evict_fn, wpool, pspool, wk=32,
                      k_order=None):
            for ob in range(LINS[wname][1] // OBW):
                _emit_ob(wname, ob, rhs_fn, evict_fn, wpool, pspool, wk,
                         k_order)

        # ---------- phase 1+2: norms, K, V, Q, f1, f2 ----------
        with tc.tile_pool(name="kvq", bufs=1) as kvq:
            KT = kvq.tile([128, KC, KEYS], BF16, tag="KT")
            V_sb = kvq.tile([128, H, H, HD + 1], FP8, tag="V")
            QT_z = kvq.tile([128, H, TOK], BF16, tag="QT")


            with (
                tc.tile_pool(name="tmp12", bufs=1) as tmp12,
                tc.tile_pool(name="wpool12", bufs=2) as wpool12,
                tc.tile_pool(name="ps_lin", bufs=3, space="PSUM") as ps_lin,
            ):
                zn_own = tmp12.tile([128, KC, TOK], FP8, tag="zn_own")
                fh = tmp12.tile([128, 2 * KC, TOK], FP8, tag="fh")

                dzl_b8 = mlp.tile([128, KC, TOK], FP8, tag="dzl8")

                norm_scope = ExitStack()
                nrm = norm_scope.enter_context(
                    tc.tile_pool(name="nrm", bufs=1))
                xrawp = norm_scope.enter_context(
                    tc.tile_pool(name="xraw", bufs=3))
                ps_ss = norm_scope.enter_context(
                    tc.tile_pool(name="ps_ss", bufs=2, space="PSUM"))

                def evict_silu(dst, ps, bias_ap):
                    # silu(x) = x * sigmoid(x), x = DS*ps + b  (b == 0 here;
                    # the linear term drops it, the sigmoid keeps it).
                    # dst/ps are [128, 2, TOK] out-block pairs.
                    sg = mlp.tile([128, 2, TOK], BF16, tag="sg", bufs=2)
                    nc.scalar.activation(sg[:], ps, AF.Sigmoid, bias=bias_ap,
                                         scale=DS)
                    nc.vector.scalar_tensor_tensor(
                        dst, ps, DS, sg[:], op0=ALU.mult, op1=ALU.mult)

                def norm_block(xT_d, w_t, dst):
                    # normed fp8 chunks into dst [128, KC, TOK].
                    # pair-granular: one DMA per 2 chunks, one copy, one
                    # fp8 square, one DoubleRow ss pass -- short latency
                    # chain, few semaphore hops.
                    ss = ps_ss.tile([1, TOK], F32, tag="ss")
                    for kp in range(KC // 2):
                        sq = xrawp.tile([128, 2, TOK], FP8, tag="sq",
                                        bufs=2)
                        xf = xrawp.tile([128, 2, TOK], F32, tag="xf",
                                        bufs=3)
                        nc.sync.dma_start(out=xf[:], in_=xT_d[kp])
                        dpair = dst[:, 2 * kp:2 * kp + 2, :]
                        nc.vector.tensor_copy(dpair, xf[:])
                        if kp % 2 == 0:
                            nc.scalar.activation(sq[:], dpair, AF.Square)
                        else:
                            nc.vector.tensor_mul(sq[:], dpair, dpair)
                        nc.tensor.matmul(ss[:], ones2[:, :, 0:1], sq[:],
                                         start=(kp == 0),
                                         stop=(kp == KC // 2 - 1),
                                         perf_mode=DR)
                    sf = xrawp.tile([1, TOK], F32, tag="sf", bufs=1)
                    nc.scalar.activation(sf[:], ss[:], AF.Sqrt,
                                         bias=eps1[:], scale=1.0 / D)
                    rcp = xrawp.tile([1, 2, TOK], F32, tag="rcp", bufs=1)
                    nc.vector.reciprocal_approx_accurate(
                        rcp[0:1, 0, :], sf[:], rcp[0:1, 1, :])
                    bc = xrawp.tile([128, TOK], F32, tag="bc", bufs=1)
                    nc.gpsimd.partition_broadcast(bc[:], rcp[0:1, 0, :])
                    for k in range(KC):
                        nc.vector.scalar_tensor_tensor(
                            dst[:, k, :], bc[:], w_t[:, k:k + 1],
                            dst[:, k, :], op0=ALU.mult, op1=ALU.mult)

                for tb in range(NTB):
                    cols = slice(tb * TOK, (tb + 1) * TOK)
                    zslices = [
                        d_in["zT"][2 * kp * 128:(2 * kp + 2) * 128, cols]
                        .rearrange("(two p) t -> p two t", p=128)
                        for kp in range(KC // 2)]
                    cslices = [
                        d_in["cT"][2 * kp * 128:(2 * kp + 2) * 128, cols]
                        .rearrange("(two p) t -> p two t", p=128)
                        for kp in range(KC // 2)]
                    if tb == 0:
                        zn_tb, cn_tb = zn_own, cn_own
                    else:
                        zn_tb = nrm.tile([128, KC, TOK], FP8, tag="zn_tb",
                                         bufs=2)
                        cn_tb = nrm.tile([128, KC, TOK], FP8, tag="cn_tb",
                                         bufs=2)
                    norm_block(zslices, wcol["wz"], zn_tb)

                    if tb == 0:
                        load_biases(["fb1"])
                    elif tb == 2:
                        load_biases(["fb2", "gb1", "gb2",
                                     "cub1", "cub2", "mb1", "mb2"])

                    # V first: needs only zn, overlaps the c-norm
                    for kc4 in range(4):
                        kcg = tb * 4 + kc4
                        ps = ps_lin.tile([128, 2, 512], F32, tag="lin")
                        for k in range(KC // 2):
                            lhs = zn_tb[:, 2 * k:2 * k + 2,
                                        kc4 * 128:(kc4 + 1) * 128]
                            for vb in range(2):
                                nc.tensor.matmul(
                                    ps[:, vb, :], lhs,
                                    vw[:, 2 * k:2 * k + 2,
                                       vb * 512:(vb + 1) * 512],
                                    start=(k == 0),
                                    stop=(k == KC // 2 - 1),
                                    perf_mode=DR)
                        for vb in range(2):
                            src = ps[:, vb, :].rearrange(
                                "p (h d) -> p h d", h=8)
                            nc.scalar.activation(
                                V_sb[:, kcg, vb * 8:(vb + 1) * 8, 0:HD],
                                src, AF.Copy, scale=DS)

                    norm_block(cslices, wcol["wc"], cn_tb)

                    if tb == 0:
                        # gpsimd is free here; QT zeros must beat Q's evicts
                        nc.gpsimd.memset(QT_z[:], 0.0)

                        # f1 needs only zn_own: fills the tensor queue
                        # while the c-norm tail chain completes
                        def f1_evict(ob, ps):
                            evict_silu(fh[:, 2 * ob:2 * ob + 2, :], ps,
                                       bias["fb1"][:, 2 * ob:2 * ob + 1])

                        linear_fm("fw1",
                                  lambda k: zn_own[:, 2 * k:2 * k + 2, :],
                                  f1_evict, wpool12, ps_lin, wk=16)

                    # K for this token block -> KT[:, :, tb]
                    def k_rhs2(k):
                        return (zn_tb[:, 2 * k:2 * k + 2, :] if k < KC // 2
                                else cn_tb[:, 2 * k - KC:2 * k - KC + 2, :])

                    def k_evict(ob, ps):
                        nc.scalar.activation(KT[:, 2 * ob:2 * ob + 2, cols],
                                             ps, AF.Copy, scale=DS)

                    linear_fm("kw", k_rhs2, k_evict, wpool12, ps_lin, wk=16)

                    if tb == 0:
                        # Q projection (own tokens), zero-padded per head
                        def q_rhs2(k):
                            return (zn_own[:, 2 * k:2 * k + 2, :]
                                    if k < KC // 2
                                    else cn_own[:, 2 * k - KC:
                                                2 * k - KC + 2, :])

                        def q_evict(ob, ps):
                            for j in range(2):
                                mc = 2 * ob + j
                                nc.scalar.activation(
                                    QT_z[0:64, 2 * mc, :], ps[0:64, j, :],
                                    AF.Copy, scale=DS)
                                nc.scalar.activation(
                                    QT_z[64:128, 2 * mc + 1, :],
                                    ps[64:128, j, :], AF.Copy, scale=DS)

                        linear_fm("qw", q_rhs2, q_evict, wpool12, ps_lin,
                                  wk=16)

                norm_scope.close()

                nc.gpsimd.memset(V_sb[:, :, :, HD:HD + 1], 1.0)

                # f2: dz_local (fp8; the extra quantization on dz is
                # attenuated by dt=0.1 downstream)
                def f2_evict(ob, ps):
                    nc.vector.tensor_scalar_mul(
                        dzl_b8[:, 2 * ob:2 * ob + 2, :], ps, DS)

                linear_fm("fw2", lambda k: fh[:, 2 * k:2 * k + 2, :],
                          f2_evict, wpool12, ps_lin, wk=16)

                # prefetch raw z/c (own block) during attention on the
                # gpsimd queue, staging fp32 -> bf16 on gpsimd
                z32 = mlp.tile([128, KC, TOK], BF16, tag="z32")
                c32 = mlp.tile([128, KC, TOK], BF16, tag="c32")
                c8 = mlp2.tile([128, KC, TOK], FP8, tag="c8")
                for k in range(KC):
                    stga = mlp.tile([128, TOK], F32, tag="zcstage", bufs=2)
                    nc.gpsimd.dma_start(
                        out=stga[:],
                        in_=d_in["zT"][k * 128:(k + 1) * 128, 0:TOK])
                    nc.gpsimd.tensor_copy(z32[:, k, :], stga[:])
                    stgb = mlp.tile([128, TOK], F32, tag="zcstage", bufs=2)
                    nc.gpsimd.dma_start(
                        out=stgb[:],
                        in_=d_in["cT"][k * 128:(k + 1) * 128, 0:TOK])
                    nc.gpsimd.tensor_copy(c32[:, k, :], stgb[:])
                    nc.gpsimd.tensor_copy(c8[:, k, :], stgb[:])

            # ---------- phase 3: sigmoid attention + g1/g2/o ----------
            gh = mlp2.tile([128, KC, TOK], FP8, tag="gh")
            s_b = mlp2.tile([128, KC, TOK], BF16, tag="s_b")
            z1_b = mlp2.tile([128, KC, TOK], BF16, tag="z1b")

            with (
                tc.tile_pool(name="rel", bufs=1) as relp,
                tc.tile_pool(name="att_s", bufs=2) as attsp,
                tc.tile_pool(name="wpool_att", bufs=2) as wpool_att,
                tc.tile_pool(name="ps_g", bufs=1, space="PSUM") as ps_g,
            ):
                att_scope = ExitStack()
                ps_sc = att_scope.enter_context(
                    tc.tile_pool(name="ps_sc", bufs=2, space="PSUM"))
                ps_av = att_scope.enter_context(
                    tc.tile_pool(name="ps_av", bufs=2, space="PSUM"))

                # gh = tanh(cat(cn, dzl) @ g_w1.T + gb1), interleaved into
                # the attention loop (tensor slack; phase is sigmoid-bound)
                def g1_evict(ob, ps):
                    nc.scalar.activation(gh[:, 2 * ob:2 * ob + 2, :], ps,
                                         AF.Tanh,
                                         bias=bias["gb1"][:, 2 * ob:2 * ob + 1],
                                         scale=DS)

                def g1_rhs(k):
                    return (cn_own[:, 2 * k:2 * k + 2, :] if k < KC // 2
                            else dzl_b8[:, 2 * k - KC:2 * k - KC + 2, :])

                for h in range(H):
                    rel = relp.tile([128, H, TOK], FP8, tag="rel", bufs=2)
                    for kc2 in range(H // 2):
                        sc = ps_sc.tile([128, 2, TOK], F32, tag="sc")
                        for j in range(2):
                            kc = 2 * kc2 + j
                            nc.tensor.matmul(
                                sc[:, j, :],
                                KT[:, h // 2, kc * 128:(kc + 1) * 128],
                                QT_z[:, h, :], start=True, stop=True)
                        nc.scalar.activation(
                            rel[:, 2 * kc2:2 * kc2 + 2, :], sc[:],
                            AF.Sigmoid, scale=sig_scale)
                    av = ps_av.tile([65, TOK], F32, tag="av")
                    for kc in range(H // 2):
                        nc.tensor.matmul(av[:],
                                         V_sb[:, 2 * kc:2 * kc + 2, h, :],
                                         rel[:, 2 * kc:2 * kc + 2, :],
                                         start=(kc == 0),
                                         stop=(kc == H // 2 - 1),
                                         perf_mode=DR)
                    # attn = av / max(rel_sum, 1)
                    rs = attsp.tile([1, 3, TOK], F32, tag="rs")
                    nc.vector.tensor_scalar_max(rs[0:1, 0, :],
                                                av[64:65, :], 1.0)
                    nc.vector.reciprocal_approx_accurate(
                        rs[0:1, 1, :], rs[0:1, 0, :], rs[0:1, 2, :])
                    bcv = attsp.tile([64, TOK], F32, tag="bcv")
                    nc.gpsimd.partition_broadcast(bcv[:], rs[0:1, 1, :])
                    po = (h % 2) * 64
                    nc.vector.tensor_mul(attnT[po:po + 64, h // 2, :],
                                         av[0:64, :], bcv[:])

                    if h in (3, 6, 9, 12):
                        _emit_ob("gw1", (h - 3) // 3, g1_rhs, g1_evict,
                                 wpool_att, ps_g, 16)

                # heads done: release score/av psum, open a rotation pool
                # for g2 + o (fills the tail while head 15's attnT lands)
                att_scope.close()
                with tc.tile_pool(name="ps_go", bufs=2,
                                  space="PSUM") as ps_go:
                    # s = dzl + gh @ g_w2.T   (dz = dt*s)
                    def g2_evict(ob, ps):
                        nc.vector.scalar_tensor_tensor(
                            s_b[:, 2 * ob:2 * ob + 2, :], ps, DS,
                            dzl_b8[:, 2 * ob:2 * ob + 2, :],
                            op0=ALU.mult, op1=ALU.add)

                    linear_fm("gw2", lambda k: gh[:, 2 * k:2 * k + 2, :],
                              g2_evict, wpool_att, ps_go, wk=16)

                    # ctx = attn @ o_w.T ; z1 = z + dt*s + ctx
                    def o_evict(ob, ps):
                        t = mlp2.tile([128, 2, TOK], F32, tag="t_z1",
                                      bufs=2)
                        nc.vector.scalar_tensor_tensor(
                            t[:], ps, DS, z32[:, 2 * ob:2 * ob + 2, :],
                            op0=ALU.mult, op1=ALU.add)
                        nc.vector.scalar_tensor_tensor(
                            z1_b[:, 2 * ob:2 * ob + 2, :],
                            s_b[:, 2 * ob:2 * ob + 2, :], dt_val, t[:],
                            op0=ALU.mult, op1=ALU.add)

                    linear_fm("ow", lambda k: attnT[:, 2 * k:2 * k + 2, :],
                              o_evict, wpool_att, ps_go, wk=16)

        # ---------- phase 4: z1 norm, cu, final MLP ----------
        with (
            tc.tile_pool(name="mlp4", bufs=1) as mlp4,
            tc.tile_pool(name="wpool4", bufs=2) as wpool4,
            tc.tile_pool(name="ps_lin4", bufs=3, space="PSUM") as ps_lin4,
            tc.tile_pool(name="ps_ss2", bufs=2, space="PSUM") as ps_ss2,
        ):
            # z1n = rms(z1) * wmlp  (frees the norm chain before m1)
            z1n = mlp4.tile([128, KC, TOK], FP8, tag="z1n")
            ss = ps_ss2.tile([1, TOK], F32, tag="ss2")
            for k in range(KC):
                sq = mlp4.tile([128, TOK], BF16, tag="sq2", bufs=2)
                nc.vector.tensor_mul(sq[:], z1_b[:, k, :], z1_b[:, k, :])
                nc.tensor.matmul(ss[:], ones_col[:], sq[:],
                                 start=(k == 0), stop=(k == KC - 1))
            sf = mlp4.tile([1, 3, TOK], F32, tag="sf2")
            nc.scalar.activation(sf[0:1, 0, :], ss[:], AF.Sqrt, bias=eps1[:],
                                 scale=1.0 / D)
            nc.vector.reciprocal_approx_accurate(
                sf[0:1, 1, :], sf[0:1, 0, :], sf[0:1, 2, :])
            bc2 = mlp4.tile([128, TOK], F32, tag="bc2")
            nc.gpsimd.partition_broadcast(bc2[:], sf[0:1, 1, :])
            for k in range(KC):
                nc.vector.scalar_tensor_tensor(
                    z1n[:, k, :], bc2[:], wcol["wmlp"][:, k:k + 1],
                    z1_b[:, k, :], op0=ALU.mult, op1=ALU.mult)

            # cu: du = silu(cat(c, z1, dt*s) @ cu_w1.T + cub1), fp8 DR.
            # fp8 stages of c/z1/s copied on gpsimd (idle here).
            z1_8 = mlp4.tile([128, KC, TOK], FP8, tag="z1_8")
            s8 = mlp4.tile([128, KC, TOK], FP8, tag="s8")
            for k in range(KC):
                nc.gpsimd.tensor_copy(s8[:, k, :], s_b[:, k, :])
                nc.gpsimd.tensor_copy(z1_8[:, k, :], z1_b[:, k, :])
            du = mlp4.tile([128, 32, TOK], BF16, tag="hid")

            def cu1_rhs(kp):
                if kp < KC // 2:
                    return c8[:, 2 * kp:2 * kp + 2, :]
                if kp < KC:
                    return z1_8[:, 2 * kp - KC:2 * kp - KC + 2, :]
                return s8[:, 2 * kp - 2 * KC:2 * kp - 2 * KC + 2, :]

            def cu1_evict(ob, ps):
                evict_silu(du[:, 2 * ob:2 * ob + 2, :], ps,
                           bias["cub1"][:, 2 * ob:2 * ob + 1])

            # pair order: conn, dz (ready early), z1 last
            cu1_korder = (list(range(0, KC // 2))
                          + list(range(KC, 3 * KC // 2))
                          + list(range(KC // 2, KC)))
            linear_fm("cuw1", cu1_rhs, cu1_evict, wpool4, ps_lin4,
                      k_order=cu1_korder)

            # mh = silu(z1n @ m_w1.T + mb1), fp8 DR
            mh = mlp4.tile([128, 32, TOK], BF16, tag="hid2")

            def m1_evict(ob, ps):
                evict_silu(mh[:, 2 * ob:2 * ob + 2, :], ps,
                           bias["mb1"][:, 2 * ob:2 * ob + 1])

            linear_fm("mw1", lambda k: z1n[:, 2 * k:2 * k + 2, :],
                      m1_evict, wpool4, ps_lin4)

            # conn_new = c + du @ cu_w2.T  (after m1: du long since ready,
            # and m2's wait on mh's tail hides under cu2)
            def cu2_evict(ob, ps):
                co = outp.tile([128, 2, TOK], F32, tag="co")
                nc.vector.scalar_tensor_tensor(
                    co[:], ps, bias["cub2"][:, 2 * ob:2 * ob + 1],
                    c32[:, 2 * ob:2 * ob + 2, :],
                    op0=ALU.add, op1=ALU.add)
                nc.sync.dma_start(
                    out=connT_d[ob * 256:(ob + 1) * 256, :]
                    .rearrange("(two p) t -> p two t", p=128), in_=co[:])

            linear_fm("cuw2", lambda k: du[:, k, :],
                      cu2_evict, wpool4, ps_lin4)

            # z2 = z1 + mh @ m_w2.T
            def m2_evict(ob, ps):
                zo = outp.tile([128, 2, TOK], F32, tag="zo")
                nc.vector.scalar_tensor_tensor(
                    zo[:], ps, bias["mb2"][:, 2 * ob:2 * ob + 1],
                    z1_b[:, 2 * ob:2 * ob + 2, :],
                    op0=ALU.add, op1=ALU.add)
                nc.sync.dma_start(
                    out=z2T_d[ob * 256:(ob + 1) * 256, :]
                    .rearrange("(two p) t -> p two t", p=128), in_=zo[:])

            linear_fm("mw2", lambda k: mh[:, k, :],
                      m2_evict, wpool4, ps_lin4)


_CACHE = {}


def _pack_ob(wT, fp8):
    # wT [n_in, n_out] -> [128, nob, kcn, OBW]:
    # packed[p, ob, k, m] = wT[k*128+p, ob*OBW+m]  (*WSCALE if fp8)
    n_in, n_out = wT.shape
    kcn, nob = n_in // 128, n_out // OBW
    if fp8:
        wT = wT * WSCALE
    dt = ml_dtypes.float8_e4m3 if fp8 else ml_dtypes.bfloat16
    return np.ascontiguousarray(
        wT.reshape(kcn, 128, nob, OBW).transpose(1, 2, 0, 3)).astype(dt)


def _prep_shared(inputs):
    def t(x):
        return np.ascontiguousarray(np.asarray(x, np.float32).T)

    dt_val = float(np.asarray(inputs["dt"]))
    cu1 = np.asarray(inputs["cu_w1"], np.float32).copy()
    cu1[:, 2 * D:] *= dt_val  # fold dz = dt*s into cu_w1's dz block
    wT = {
        "fw1": t(inputs["f_w1"]), "fw2": t(inputs["f_w2"]),
        "gw1": t(inputs["g_w1"]), "gw2": t(inputs["g_w2"]),
        "qw": t(inputs["q_w"]), "kw": t(inputs["k_w"]),
        "ow": t(inputs["o_w"]),
        "cuw1": np.ascontiguousarray(cu1.T),
        "cuw2": t(inputs["cu_w2"]),
        "mw1": t(inputs["m_w1"]), "mw2": t(inputs["m_w2"]),
    }
    shared = {name + "P": _pack_ob(w, name in FP8_LINS)
              for name, w in wT.items()}
    # vw: k-major [128, kcn, n_out] (streamed as moving operand)
    vwT = t(inputs["v_w"])
    shared["vwK"] = np.ascontiguousarray(
        (vwT * WSCALE).reshape(KC, 128, D).transpose(1, 0, 2)
    ).astype(ml_dtypes.float8_e4m3)
    for name, key in [("fb1", "f_b1"), ("fb2", "f_b2"), ("gb1", "g_b1"),
                      ("gb2", "g_b2"), ("cub1", "cu_b1"), ("cub2", "cu_b2"),
                      ("mb1", "m_b1"), ("mb2", "m_b2"), ("wz", "w_z"),
                      ("wc", "w_c"), ("wmlp", "w_mlp")]:
        # [n] -> [128, n//128] with tile[p, c] = b[c*128 + p]
        arr = np.asarray(inputs[key], np.float32)
        shared[name] = np.ascontiguousarray(arr.reshape(-1, 128).T)
    return shared


def kernel(**inputs):
    z = np.asarray(inputs["z"], np.float32)
    conn = np.asarray(inputs["connection"], np.float32)
    dt_val = float(np.asarray(inputs["dt"]))
    temp_val = float(np.asarray(inputs["temp"]))

    key = (dt_val, temp_val)
    if key not in _CACHE:
        _CACHE[key] = build_program(dt_val, temp_val)
    nc = _CACHE[key]

    shared = _prep_shared(inputs)
    zT = [np.ascontiguousarray(z[b].T) for b in range(B)]
    cT = [np.ascontiguousarray(conn[b].T) for b in range(B)]

    in_maps = []
    for c in range(NCORES):
        b, tb = divmod(c, NTB)
        m = dict(shared)
        m["zT"] = np.ascontiguousarray(np.roll(zT[b], -tb * TOK, axis=1))
        m["cT"] = np.ascontiguousarray(np.roll(cT[b], -tb * TOK, axis=1))
        in_maps.append(m)

    res = run_bass_kernel_spmd(nc, in_maps, list(range(NCORES)))

    z2 = np.empty((B, L, D), np.float32)
    conn_new = np.empty((B, L, D), np.float32)
    for c in range(NCORES):
        b, tb = divmod(c, NTB)
        sl = slice(tb * TOK, (tb + 1) * TOK)
        z2[b, sl, :] = res.results[c]["z2T"].T
        conn_new[b, sl, :] = res.results[c]["connT"].T
    return z2, conn_new, z


# revision 24
# speedup vs baseline: 1.0324x; 1.0324x over previous
"""CovariantEvolutionBlock Trainium2 kernel (v2).

Strategy: token-parallel over B*L across 8 cores (512 tokens/core), zero
collectives. Each core recomputes full-batch K/V for attention (inputs are
rotated per-core so "own" tokens are always columns 0:512; sigmoid attention
is permutation-invariant over keys). Activations are kept feature-major
[dims, tokens] on-chip so matmul chains need no transposes.

All dense linears except cu*/m* run in fp8(e4m3) with DoubleRow perf mode;
weights are host-scaled by 256 to escape the e4m3 subnormal range and
descaled inside the PSUM evictions. cu/m stay bf16 (fp8 there costs ~2.4e-2
rel err, over the 2e-2 gate).

v2 scheduling changes vs v1:
- DMAs spread across engine queues (weights/biases on scalar, vw + z32/c32
  prefetch on gpsimd, activations+slabs on sync); big memsets on gpsimd.
- rms square-sums run as fp8 DoubleRow matmuls against a [128,2,1] ones
  stationary (half the passes), with x^2 computed fp8*fp8 on vector.
- g1 interleaved into the attention loop (tensor has slack there: the phase
  is scalar-sigmoid-bound), g2+o right after the head loop; phase4 is
  cu1 -> m1 -> cu2 -> m2 with cu1's contraction reordered (conn, dz first,
  z1 last) so each linear starts before its predecessor's tail lands.
- weight slabs prefetch 2-deep from a pool opened before attention.
- raw z/c staged per-chunk to bf16 (halves their SBUF) -- z1/conn_new pick
  up ~0.3% extra rel err from the bf16 carry, well within budget.
"""

import sys

try:
    import concourse.bass as bass  # noqa: F401
except ImportError:
    sys.path.insert(0, "/opt/trn_rl_repo")

import numpy as np
import ml_dtypes

import concourse.bacc as bacc
import concourse.tile as tile
import concourse.mybir as mybir
from concourse.bass_utils import run_bass_kernel_spmd

F32 = mybir.dt.float32
BF16 = mybir.dt.bfloat16
FP8 = mybir.dt.float8e4
AF = mybir.ActivationFunctionType
ALU = mybir.AluOpType
DR = mybir.MatmulPerfMode.DoubleRow

B, L, D, H, HD = 2, 2048, 1024, 16, 64
EPS = 1e-6
NCORES = 8
TOK = 512          # own tokens per core
KEYS = 2048        # keys per batch
KC = D // 128      # 8 feature chunks of 128
NTB = KEYS // TOK  # 4 token blocks per batch
OBW = 256          # out-block width (2 m-chunks) per psum tile
WSCALE = 256.0     # fp8 weight scale (weights ~N(0,0.02) are subnormal raw)
DS = 1.0 / WSCALE

# name -> (n_in, n_out) for ob-major packed linears
LINS = {
    "fw1": (D, 2 * D), "fw2": (2 * D, D),
    "gw1": (2 * D, D), "gw2": (D, D),
    "qw": (2 * D, D), "kw": (2 * D, D),
    "ow": (D, D),
    "cuw1": (3 * D, 2 * D), "cuw2": (2 * D, D),
    "mw1": (D, 4 * D), "mw2": (4 * D, D),
}
# cu/m weight+act quantization lands unattenuated on the outputs
# (conn_new = c + cu2(...), z2 = z1 + m2(...)): fp8 there measures ~2.4e-2
# end-to-end, over the 2e-2 gate. Everything else is attenuated (dt=0.1 on
# dz, 1/rel_sum on ctx) -> fp8 ok.
FP8_LINS = {"fw1", "fw2", "gw1", "gw2", "qw", "kw", "ow", "cuw1", "mw1"}


def _bias_ap(dram_ap):
    # [dim] -> [128, dim//128]: tile[p, c] = bias[c*128 + p]
    return dram_ap.rearrange("(c p) -> p c", p=128)


def build_program(dt_val: float, temp_val: float):
    nc = bacc.Bacc("TRN2", target_bir_lowering=False, debug=False,
                   num_devices=NCORES)

    d_in = {}
    for name, shape, dt in [
        ("zT", [D, KEYS], F32), ("cT", [D, KEYS], F32),
        ("vwK", [128, KC, D], FP8),  # k-major: streamed operand
        ("fb1", [128, 16], F32), ("fb2", [128, 8], F32),
        ("gb1", [128, 8], F32), ("gb2", [128, 8], F32),
        ("cub1", [128, 16], F32), ("cub2", [128, 8], F32),
        ("mb1", [128, 32], F32), ("mb2", [128, 8], F32),
        ("wz", [128, 8], F32), ("wc", [128, 8], F32),
        ("wmlp", [128, 8], F32),
    ]:
        d_in[name] = nc.dram_tensor(name, shape, dt, kind="ExternalInput").ap()
    for name, (n_in, n_out) in LINS.items():
        wdt = FP8 if name in FP8_LINS else BF16
        d_in[name + "P"] = nc.dram_tensor(
            name + "P", [128, n_out // OBW, n_in // 128, OBW], wdt,
            kind="ExternalInput").ap()

    z2T_d = nc.dram_tensor("z2T", [D, TOK], F32, kind="ExternalOutput").ap()
    connT_d = nc.dram_tensor("connT", [D, TOK], F32, kind="ExternalOutput").ap()

    sig_scale = float(temp_val) * (HD ** -0.5)

    with tile.TileContext(nc) as tc:
        _emit(nc, tc, d_in, z2T_d, connT_d, float(dt_val), sig_scale)
    nc.compile()
    return nc


def _emit(nc, tc, d_in, z2T_d, connT_d, dt_val, sig_scale):
    from contextlib import ExitStack

    ctx = ExitStack()
    with ctx:
        # ---------- persistent pools ----------
        const = ctx.enter_context(tc.tile_pool(name="const", bufs=1))
        persist = ctx.enter_context(tc.tile_pool(name="persist", bufs=1))

        # rms weights + ones/eps needed immediately; weight/bias DMAs ride
        # the scalar queue so sync is free for the z/c activation chunks.
        # The V-projection weight goes first: V's matmuls need it ~12us in.
        vw_pool = ctx.enter_context(tc.tile_pool(name="vwp", bufs=1))
        vw = vw_pool.tile([128, KC, D], FP8, tag="vw")
        nc.scalar.dma_start(out=vw[:], in_=d_in["vwK"][:, :, :])
        wcol = {}
        for name in ["wz", "wc", "wmlp"]:
            t = const.tile([128, KC], F32, tag=name)
            nc.scalar.dma_start(out=t[:], in_=d_in[name][:, :])
            wcol[name] = t
        ones_col = const.tile([128, 1], BF16, tag="ones")
        nc.vector.memset(ones_col[:], 1.0)
        # [128,2,16] so the DR chunk-pair stride is 16B-aligned; used as
        # a [128,2,1] slice
        ones2 = const.tile([128, 2, 16], FP8, tag="ones2")
        nc.vector.memset(ones2[:], 1.0)
        eps1 = const.tile([1, 1], F32, tag="eps1")
        nc.vector.memset(eps1[:], EPS)
        warm = const.tile([1, 2], F32, tag="warm")
        nc.scalar.activation(warm[0:1, 0:1], eps1[:], AF.Square)
        nc.scalar.activation(warm[0:1, 1:2], eps1[:], AF.Sqrt)

        # persistent activations (own tokens, feature-major, fp8)
        cn_own = persist.tile([128, KC, TOK], FP8, tag="cn_own")
        attnT = persist.tile([128, KC, TOK], FP8, tag="attnT")
        mlp = ctx.enter_context(tc.tile_pool(name="mlp", bufs=1))
        mlp2 = ctx.enter_context(tc.tile_pool(name="mlp2", bufs=1))
        outp = ctx.enter_context(tc.tile_pool(name="outp", bufs=1))

        bias = {}

        def load_biases(names):
            for name in names:
                n = d_in[name].shape[1]
                t = const.tile([128, n], F32, tag=name)
                nc.scalar.dma_start(out=t[:], in_=d_in[name][:, :])
                bias[name] = t

        # ---------- generic feature-major linear ----------
        # Weights stream as one packed slab per 256-wide out-block.
        # fp8 linears use DoubleRow (rhs_fn returns [128,2,TOK] pairs);
        # bf16 linears use plain matmul (rhs_fn returns [128,TOK] chunks).
        def _emit_ob(wname, ob, rhs_fn, evict_fn, wpool, pspool, wk,
                     korder=None):
            n_in, n_out = LINS[wname]
            fp8 = wname in FP8_LINS
            wdt = FP8 if fp8 else BF16
            kcn = n_in // 128
            ksteps = kcn // 2 if fp8 else kcn
            ks = list(range(ksteps)) if korder is None else korder
            wP = d_in[wname + "P"]
            w = wpool.tile([128, wk, OBW], wdt, tag="wslab")
            nc.sync.dma_start(out=w[:, 0:kcn, :], in_=wP[:, ob, :, :])
            ps = pspool.tile([128, 2, 512], F32, tag="lin")
            for i, k in enumerate(ks):
                for m in range(2):
                    if fp8:
                        nc.tensor.matmul(
                            ps[:, m, :TOK],
                            w[:, 2 * k:2 * k + 2, m * 128:(m + 1) * 128],
                            rhs_fn(k), start=(i == 0),
                            stop=(i == ksteps - 1), perf_mode=DR)
                    else:
                        nc.tensor.matmul(
                            ps[:, m, :TOK],
                            w[:, k, m * 128:(m + 1) * 128],
                            rhs_fn(k), start=(i == 0),
                            stop=(i == ksteps - 1))
            evict_fn(ob, ps[:, :, :TOK])

        def linear_fm(wname, rhs_fn, evict_fn, wpool, pspool, wk=32,
                      k_order=None):
            for ob in range(LINS[wname][1] // OBW):
                _emit_ob(wname, ob, rhs_fn, evict_fn, wpool, pspool, wk,
                         k_order)

        # ---------- phase 1+2: norms, K, V, Q, f1, f2 ----------
        with tc.tile_pool(name="kvq", bufs=1) as kvq:
            KT = kvq.tile([128, KC, KEYS], BF16, tag="KT")
            V_sb = kvq.tile([128, H, H, HD + 1], FP8, tag="V")
            QT_z = kvq.tile([128, H, TOK], BF16, tag="QT")


            with (
                tc.tile_pool(name="tmp12", bufs=1) as tmp12,
                tc.tile_pool(name="wpool12", bufs=2) as wpool12,
                tc.tile_pool(name="ps_lin", bufs=3, space="PSUM") as ps_lin,
            ):
                zn_own = tmp12.tile([128, KC, TOK], FP8, tag="zn_own")
                fh = tmp12.tile([128, 2 * KC, TOK], FP8, tag="fh")

                dzl_b8 = mlp.tile([128, KC, TOK], FP8, tag="dzl8")

                norm_scope = ExitStack()
                nrm = norm_scope.enter_context(
                    tc.tile_pool(name="nrm", bufs=1))
                xrawp = norm_scope.enter_context(
                    tc.tile_pool(name="xraw", bufs=3))
                ps_ss = norm_scope.enter_context(
                    tc.tile_pool(name="ps_ss", bufs=2, space="PSUM"))

                def evict_silu(dst, ps, bias_ap):
                    # silu(x) = x * sigmoid(x), x = DS*ps + b  (b == 0 here;
                    # the linear term drops it, the sigmoid keeps it).
                    # dst/ps are [128, 2, TOK] out-block pairs.
                    sg = mlp.tile([128, 2, TOK], BF16, tag="sg", bufs=2)
                    nc.scalar.activation(sg[:], ps, AF.Sigmoid, bias=bias_ap,
                                         scale=DS)
                    nc.vector.scalar_tensor_tensor(
                        dst, ps, DS, sg[:], op0=ALU.mult, op1=ALU.mult)

                def norm_block(xT_d, w_t, dst):
                    # normed fp8 chunks into dst [128, KC, TOK].
                    # pair-granular: one DMA per 2 chunks, one copy, one
                    # fp8 square, one DoubleRow ss pass -- short latency
                    # chain, few semaphore hops.
                    ss = ps_ss.tile([1, TOK], F32, tag="ss")
                    for kp in range(KC // 2):
                        sq = xrawp.tile([128, 2, TOK], FP8, tag="sq",
                                        bufs=2)
                        xf = xrawp.tile([128, 2, TOK], F32, tag="xf",
                                        bufs=3)
                        nc.sync.dma_start(out=xf[:], in_=xT_d[kp])
                        dpair = dst[:, 2 * kp:2 * kp + 2, :]
                        nc.vector.tensor_copy(dpair, xf[:])
                        if kp % 2 == 0:
                            nc.scalar.activation(sq[:], dpair, AF.Square)
                        else:
                            nc.vector.tensor_mul(sq[:], dpair, dpair)
                        nc.tensor.matmul(ss[:], ones2[:, :, 0:1], sq[:],
                                         start=(kp == 0),
                                         stop=(kp == KC // 2 - 1),
                                         perf_mode=DR)
                    sf = xrawp.tile([1, TOK], F32, tag="sf", bufs=1)
                    nc.scalar.activation(sf[:], ss[:], AF.Sqrt,
                                         bias=eps1[:], scale=1.0 / D)
                    rcp = xrawp.tile([1, 2, TOK], F32, tag="rcp", bufs=1)
                    nc.vector.reciprocal_approx_accurate(
                        rcp[0:1, 0, :], sf[:], rcp[0:1, 1, :])
                    bc = xrawp.tile([128, TOK], F32, tag="bc", bufs=1)
                    nc.gpsimd.partition_broadcast(bc[:], rcp[0:1, 0, :])
                    for k in range(KC):
                        nc.vector.scalar_tensor_tensor(
                            dst[:, k, :], bc[:], w_t[:, k:k + 1],
                            dst[:, k, :], op0=ALU.mult, op1=ALU.mult)

                for tb in range(NTB):
                    cols = slice(tb * TOK, (tb + 1) * TOK)
                    zslices = [
                        d_in["zT"][2 * kp * 128:(2 * kp + 2) * 128, cols]
                        .rearrange("(two p) t -> p two t", p=128)
                        for kp in range(KC // 2)]
                    cslices = [
                        d_in["cT"][2 * kp * 128:(2 * kp + 2) * 128, cols]
                        .rearrange("(two p) t -> p two t", p=128)
                        for kp in range(KC // 2)]
                    if tb == 0:
                        zn_tb, cn_tb = zn_own, cn_own
                    else:
                        zn_tb = nrm.tile([128, KC, TOK], FP8, tag="zn_tb",
                                         bufs=2)
                        cn_tb = nrm.tile([128, KC, TOK], FP8, tag="cn_tb",
                                         bufs=2)
                    norm_block(zslices, wcol["wz"], zn_tb)

                    if tb == 0:
                        load_biases(["fb1"])
                    elif tb == 2:
                        load_biases(["fb2", "gb1", "gb2",
                                     "cub1", "cub2", "mb1", "mb2"])

                    # V first: needs only zn, overlaps the c-norm
                    for kc4 in range(4):
                        kcg = tb * 4 + kc4
                        ps = ps_lin.tile([128, 2, 512], F32, tag="lin")
                        for k in range(KC // 2):
                            lhs = zn_tb[:, 2 * k:2 * k + 2,
                                        kc4 * 128:(kc4 + 1) * 128]
                            for vb in range(2):
                                nc.tensor.matmul(
                                    ps[:, vb, :], lhs,
                                    vw[:, 2 * k:2 * k + 2,
                                       vb * 512:(vb + 1) * 512],
                                    start=(k == 0),
                                    stop=(k == KC // 2 - 1),
                                    perf_mode=DR)
                        for vb in range(2):
                            src = ps[:, vb, :].rearrange(
                                "p (h d) -> p h d", h=8)
                            nc.scalar.activation(
                                V_sb[:, kcg, vb * 8:(vb + 1) * 8, 0:HD],
                                src, AF.Copy, scale=DS)

                    norm_block(cslices, wcol["wc"], cn_tb)

                    if tb == 0:
                        # gpsimd is free here; QT zeros must beat Q's evicts
                        nc.gpsimd.memset(QT_z[:], 0.0)

                        # f1 needs only zn_own: fills the tensor queue
                        # while the c-norm tail chain completes
                        def f1_evict(ob, ps):
                            evict_silu(fh[:, 2 * ob:2 * ob + 2, :], ps,
                                       bias["fb1"][:, 2 * ob:2 * ob + 1])

                        linear_fm("fw1",
                                  lambda k: zn_own[:, 2 * k:2 * k + 2, :],
                                  f1_evict, wpool12, ps_lin, wk=16)

                    # K for this token block -> KT[:, :, tb]
                    def k_rhs2(k):
                        return (zn_tb[:, 2 * k:2 * k + 2, :] if k < KC // 2
                                else cn_tb[:, 2 * k - KC:2 * k - KC + 2, :])

                    def k_evict(ob, ps):
                        nc.scalar.activation(KT[:, 2 * ob:2 * ob + 2, cols],
                                             ps, AF.Copy, scale=DS)

                    linear_fm("kw", k_rhs2, k_evict, wpool12, ps_lin, wk=16)

                    if tb == 0:
                        # Q projection (own tokens), zero-padded per head
                        def q_rhs2(k):
                            return (zn_own[:, 2 * k:2 * k + 2, :]
                                    if k < KC // 2
                                    else cn_own[:, 2 * k - KC:
                                                2 * k - KC + 2, :])

                        def q_evict(ob, ps):
                            for j in range(2):
                                mc = 2 * ob + j
                                nc.scalar.activation(
                                    QT_z[0:64, 2 * mc, :], ps[0:64, j, :],
                                    AF.Copy, scale=DS)
                                nc.scalar.activation(
                                    QT_z[64:128, 2 * mc + 1, :],
                                    ps[64:128, j, :], AF.Copy, scale=DS)

                        linear_fm("qw", q_rhs2, q_evict, wpool12, ps_lin,
                                  wk=16)

                norm_scope.close()

                nc.gpsimd.memset(V_sb[:, :, :, HD:HD + 1], 1.0)

                # f2: dz_local (fp8; the extra quantization on dz is
                # attenuated by dt=0.1 downstream)
                def f2_evict(ob, ps):
                    nc.vector.tensor_scalar_mul(
                        dzl_b8[:, 2 * ob:2 * ob + 2, :], ps, DS)

                linear_fm("fw2", lambda k: fh[:, 2 * k:2 * k + 2, :],
                          f2_evict, wpool12, ps_lin, wk=16)

                # prefetch raw z/c (own block) during attention on the
                # gpsimd queue, staging fp32 -> bf16 on gpsimd
                z32 = mlp.tile([128, KC, TOK], BF16, tag="z32")
                c32 = mlp.tile([128, KC, TOK], BF16, tag="c32")
                c8 = mlp2.tile([128, KC, TOK], FP8, tag="c8")
                # staging DMAs ride the sync queue: their readiness-at-t0
                # otherwise hoists the whole chain into tb0's gpsimd
                # program, starving the norm broadcasts there. The copies
                # chain behind the DMAs by data deps, so they land late too.
                for k in range(KC):
                    stga = mlp.tile([128, TOK], F32, tag="zcstage", bufs=2)
                    nc.sync.dma_start(
                        out=stga[:],
                        in_=d_in["zT"][k * 128:(k + 1) * 128, 0:TOK])
                    nc.gpsimd.tensor_copy(z32[:, k, :], stga[:])
                    stgb = mlp.tile([128, TOK], F32, tag="zcstage", bufs=2)
                    nc.sync.dma_start(
                        out=stgb[:],
                        in_=d_in["cT"][k * 128:(k + 1) * 128, 0:TOK])
                    nc.gpsimd.tensor_copy(c32[:, k, :], stgb[:])
                    nc.gpsimd.tensor_copy(c8[:, k, :], stgb[:])

            # ---------- phase 3: sigmoid attention + g1/g2/o ----------
            gh = mlp2.tile([128, KC, TOK], FP8, tag="gh")
            s_b = mlp2.tile([128, KC, TOK], BF16, tag="s_b")
            z1_b = mlp2.tile([128, KC, TOK], BF16, tag="z1b")

            with (
                tc.tile_pool(name="rel", bufs=1) as relp,
                tc.tile_pool(name="att_s", bufs=2) as attsp,
                tc.tile_pool(name="wpool_att", bufs=2) as wpool_att,
                tc.tile_pool(name="ps_g", bufs=1, space="PSUM") as ps_g,
            ):
                att_scope = ExitStack()
                ps_sc = att_scope.enter_context(
                    tc.tile_pool(name="ps_sc", bufs=2, space="PSUM"))
                ps_av = att_scope.enter_context(
                    tc.tile_pool(name="ps_av", bufs=2, space="PSUM"))

                # gh = tanh(cat(cn, dzl) @ g_w1.T + gb1), interleaved into
                # the attention loop (tensor slack; phase is sigmoid-bound)
                def g1_evict(ob, ps):
                    nc.scalar.activation(gh[:, 2 * ob:2 * ob + 2, :], ps,
                                         AF.Tanh,
                                         bias=bias["gb1"][:, 2 * ob:2 * ob + 1],
                                         scale=DS)

                def g1_rhs(k):
                    return (cn_own[:, 2 * k:2 * k + 2, :] if k < KC // 2
                            else dzl_b8[:, 2 * k - KC:2 * k - KC + 2, :])

                for h in range(H):
                    rel = relp.tile([128, H, TOK], FP8, tag="rel", bufs=2)
                    for kc2 in range(H // 2):
                        sc = ps_sc.tile([128, 2, TOK], F32, tag="sc")
                        for j in range(2):
                            kc = 2 * kc2 + j
                            nc.tensor.matmul(
                                sc[:, j, :],
                                KT[:, h // 2, kc * 128:(kc + 1) * 128],
                                QT_z[:, h, :], start=True, stop=True)
                        nc.scalar.activation(
                            rel[:, 2 * kc2:2 * kc2 + 2, :], sc[:],
                            AF.Sigmoid, scale=sig_scale)
                    av = ps_av.tile([65, TOK], F32, tag="av")
                    for kc in range(H // 2):
                        nc.tensor.matmul(av[:],
                                         V_sb[:, 2 * kc:2 * kc + 2, h, :],
                                         rel[:, 2 * kc:2 * kc + 2, :],
                                         start=(kc == 0),
                                         stop=(kc == H // 2 - 1),
                                         perf_mode=DR)
                    # attn = av / max(rel_sum, 1)
                    rs = attsp.tile([1, 3, TOK], F32, tag="rs")
                    nc.vector.tensor_scalar_max(rs[0:1, 0, :],
                                                av[64:65, :], 1.0)
                    nc.vector.reciprocal_approx_accurate(
                        rs[0:1, 1, :], rs[0:1, 0, :], rs[0:1, 2, :])
                    bcv = attsp.tile([64, TOK], F32, tag="bcv")
                    nc.gpsimd.partition_broadcast(bcv[:], rs[0:1, 1, :])
                    po = (h % 2) * 64
                    nc.vector.tensor_mul(attnT[po:po + 64, h // 2, :],
                                         av[0:64, :], bcv[:])

                    if h in (3, 6, 9, 12):
                        _emit_ob("gw1", (h - 3) // 3, g1_rhs, g1_evict,
                                 wpool_att, ps_g, 16)

                # heads done: release score/av psum, open a rotation pool
                # for g2 + o (fills the tail while head 15's attnT lands)
                att_scope.close()
                with tc.tile_pool(name="ps_go", bufs=2,
                                  space="PSUM") as ps_go:
                    # s = dzl + gh @ g_w2.T   (dz = dt*s)
                    def g2_evict(ob, ps):
                        nc.vector.scalar_tensor_tensor(
                            s_b[:, 2 * ob:2 * ob + 2, :], ps, DS,
                            dzl_b8[:, 2 * ob:2 * ob + 2, :],
                            op0=ALU.mult, op1=ALU.add)

                    linear_fm("gw2", lambda k: gh[:, 2 * k:2 * k + 2, :],
                              g2_evict, wpool_att, ps_go, wk=16)

                    # ctx = attn @ o_w.T ; z1 = z + dt*s + ctx
                    def o_evict(ob, ps):
                        t = mlp2.tile([128, 2, TOK], F32, tag="t_z1",
                                      bufs=2)
                        nc.vector.scalar_tensor_tensor(
                            t[:], ps, DS, z32[:, 2 * ob:2 * ob + 2, :],
                            op0=ALU.mult, op1=ALU.add)
                        nc.vector.scalar_tensor_tensor(
                            z1_b[:, 2 * ob:2 * ob + 2, :],
                            s_b[:, 2 * ob:2 * ob + 2, :], dt_val, t[:],
                            op0=ALU.mult, op1=ALU.add)

                    linear_fm("ow", lambda k: attnT[:, 2 * k:2 * k + 2, :],
                              o_evict, wpool_att, ps_go, wk=16)

        # ---------- phase 4: z1 norm, cu, final MLP ----------
        with (
            tc.tile_pool(name="mlp4", bufs=1) as mlp4,
            tc.tile_pool(name="wpool4", bufs=2) as wpool4,
            tc.tile_pool(name="ps_lin4", bufs=3, space="PSUM") as ps_lin4,
            tc.tile_pool(name="ps_ss2", bufs=2, space="PSUM") as ps_ss2,
        ):
            # z1n = rms(z1) * wmlp  (frees the norm chain before m1)
            z1n = mlp4.tile([128, KC, TOK], FP8, tag="z1n")
            ss = ps_ss2.tile([1, TOK], F32, tag="ss2")
            for k in range(KC):
                sq = mlp4.tile([128, TOK], BF16, tag="sq2", bufs=2)
                nc.vector.tensor_mul(sq[:], z1_b[:, k, :], z1_b[:, k, :])
                nc.tensor.matmul(ss[:], ones_col[:], sq[:],
                                 start=(k == 0), stop=(k == KC - 1))
            sf = mlp4.tile([1, 3, TOK], F32, tag="sf2")
            nc.scalar.activation(sf[0:1, 0, :], ss[:], AF.Sqrt, bias=eps1[:],
                                 scale=1.0 / D)
            nc.vector.reciprocal_approx_accurate(
                sf[0:1, 1, :], sf[0:1, 0, :], sf[0:1, 2, :])
            bc2 = mlp4.tile([128, TOK], F32, tag="bc2")
            nc.gpsimd.partition_broadcast(bc2[:], sf[0:1, 1, :])
            for k in range(KC):
                nc.vector.scalar_tensor_tensor(
                    z1n[:, k, :], bc2[:], wcol["wmlp"][:, k:k + 1],
                    z1_b[:, k, :], op0=ALU.mult, op1=ALU.mult)

            # cu: du = silu(cat(c, z1, dt*s) @ cu_w1.T + cub1), fp8 DR.
            # fp8 stages of c/z1/s copied on gpsimd (idle here).
            z1_8 = mlp4.tile([128, KC, TOK], FP8, tag="z1_8")
            s8 = mlp4.tile([128, KC, TOK], FP8, tag="s8")
            for k in range(KC):
                nc.gpsimd.tensor_copy(s8[:, k, :], s_b[:, k, :])
                nc.gpsimd.tensor_copy(z1_8[:, k, :], z1_b[:, k, :])
            du = mlp4.tile([128, 32, TOK], BF16, tag="hid")

            def cu1_rhs(kp):
                if kp < KC // 2:
                    return c8[:, 2 * kp:2 * kp + 2, :]
                if kp < KC:
                    return z1_8[:, 2 * kp - KC:2 * kp - KC + 2, :]
                return s8[:, 2 * kp - 2 * KC:2 * kp - 2 * KC + 2, :]

            def cu1_evict(ob, ps):
                evict_silu(du[:, 2 * ob:2 * ob + 2, :], ps,
                           bias["cub1"][:, 2 * ob:2 * ob + 1])

            # pair order: conn, dz (ready early), z1 last
            cu1_korder = (list(range(0, KC // 2))
                          + list(range(KC, 3 * KC // 2))
                          + list(range(KC // 2, KC)))
            linear_fm("cuw1", cu1_rhs, cu1_evict, wpool4, ps_lin4,
                      k_order=cu1_korder)

            # mh = silu(z1n @ m_w1.T + mb1), fp8 DR
            mh = mlp4.tile([128, 32, TOK], BF16, tag="hid2")

            def m1_evict(ob, ps):
                evict_silu(mh[:, 2 * ob:2 * ob + 2, :], ps,
                           bias["mb1"][:, 2 * ob:2 * ob + 1])

            linear_fm("mw1", lambda k: z1n[:, 2 * k:2 * k + 2, :],
                      m1_evict, wpool4, ps_lin4)

            # conn_new = c + du @ cu_w2.T  (after m1: du long since ready,
            # and m2's wait on mh's tail hides under cu2)
            def cu2_evict(ob, ps):
                co = outp.tile([128, 2, TOK], F32, tag="co")
                nc.vector.scalar_tensor_tensor(
                    co[:], ps, bias["cub2"][:, 2 * ob:2 * ob + 1],
                    c32[:, 2 * ob:2 * ob + 2, :],
                    op0=ALU.add, op1=ALU.add)
                nc.sync.dma_start(
                    out=connT_d[ob * 256:(ob + 1) * 256, :]
                    .rearrange("(two p) t -> p two t", p=128), in_=co[:])

            linear_fm("cuw2", lambda k: du[:, k, :],
                      cu2_evict, wpool4, ps_lin4)

            # z2 = z1 + mh @ m_w2.T
            def m2_evict(ob, ps):
                zo = outp.tile([128, 2, TOK], F32, tag="zo")
                nc.vector.scalar_tensor_tensor(
                    zo[:], ps, bias["mb2"][:, 2 * ob:2 * ob + 1],
                    z1_b[:, 2 * ob:2 * ob + 2, :],
                    op0=ALU.add, op1=ALU.add)
                nc.sync.dma_start(
                    out=z2T_d[ob * 256:(ob + 1) * 256, :]
                    .rearrange("(two p) t -> p two t", p=128), in_=zo[:])

            linear_fm("mw2", lambda k: mh[:, k, :],
                      m2_evict, wpool4, ps_lin4)


_CACHE = {}


def _pack_ob(wT, fp8):
    # wT [n_in, n_out] -> [128, nob, kcn, OBW]:
    # packed[p, ob, k, m] = wT[k*128+p, ob*OBW+m]  (*WSCALE if fp8)
    n_in, n_out = wT.shape
    kcn, nob = n_in // 128, n_out // OBW
    if fp8:
        wT = wT * WSCALE
    dt = ml_dtypes.float8_e4m3 if fp8 else ml_dtypes.bfloat16
    return np.ascontiguousarray(
        wT.reshape(kcn, 128, nob, OBW).transpose(1, 2, 0, 3)).astype(dt)


def _prep_shared(inputs):
    def t(x):
        return np.ascontiguousarray(np.asarray(x, np.float32).T)

    dt_val = float(np.asarray(inputs["dt"]))
    cu1 = np.asarray(inputs["cu_w1"], np.float32).copy()
    cu1[:, 2 * D:] *= dt_val  # fold dz = dt*s into cu_w1's dz block
    wT = {
        "fw1": t(inputs["f_w1"]), "fw2": t(inputs["f_w2"]),
        "gw1": t(inputs["g_w1"]), "gw2": t(inputs["g_w2"]),
        "qw": t(inputs["q_w"]), "kw": t(inputs["k_w"]),
        "ow": t(inputs["o_w"]),
        "cuw1": np.ascontiguousarray(cu1.T),
        "cuw2": t(inputs["cu_w2"]),
        "mw1": t(inputs["m_w1"]), "mw2": t(inputs["m_w2"]),
    }
    shared = {name + "P": _pack_ob(w, name in FP8_LINS)
              for name, w in wT.items()}
    # vw: k-major [128, kcn, n_out] (streamed as moving operand)
    vwT = t(inputs["v_w"])
    shared["vwK"] = np.ascontiguousarray(
        (vwT * WSCALE).reshape(KC, 128, D).transpose(1, 0, 2)
    ).astype(ml_dtypes.float8_e4m3)
    for name, key in [("fb1", "f_b1"), ("fb2", "f_b2"), ("gb1", "g_b1"),
                      ("gb2", "g_b2"), ("cub1", "cu_b1"), ("cub2", "cu_b2"),
                      ("mb1", "m_b1"), ("mb2", "m_b2"), ("wz", "w_z"),
                      ("wc", "w_c"), ("wmlp", "w_mlp")]:
        # [n] -> [128, n//128] with tile[p, c] = b[c*128 + p]
        arr = np.asarray(inputs[key], np.float32)
        shared[name] = np.ascontiguousarray(arr.reshape(-1, 128).T)
    return shared


def kernel(**inputs):
    z = np.asarray(inputs["z"], np.float32)
    conn = np.asarray(inputs["connection"], np.float32)
    dt_val = float(np.asarray(inputs["dt"]))
    temp_val = float(np.asarray(inputs["temp"]))

    key = (dt_val, temp_val)
    if key not in _CACHE:
        _CACHE[key] = build_program(dt_val, temp_val)
    nc = _CACHE[key]

    shared = _prep_shared(inputs)
    zT = [np.ascontiguousarray(z[b].T) for b in range(B)]
    cT = [np.ascontiguousarray(conn[b].T) for b in range(B)]

    in_maps = []
    for c in range(NCORES):
        b, tb = divmod(c, NTB)
        m = dict(shared)
        m["zT"] = np.ascontiguousarray(np.roll(zT[b], -tb * TOK, axis=1))
        m["cT"] = np.ascontiguousarray(np.roll(cT[b], -tb * TOK, axis=1))
        in_maps.append(m)

    res = run_bass_kernel_spmd(nc, in_maps, list(range(NCORES)))

    z2 = np.empty((B, L, D), np.float32)
    conn_new = np.empty((B, L, D), np.float32)
    for c in range(NCORES):
        b, tb = divmod(c, NTB)
        sl = slice(tb * TOK, (tb + 1) * TOK)
        z2[b, sl, :] = res.results[c]["z2T"].T
        conn_new[b, sl, :] = res.results[c]["connT"].T
    return z2, conn_new, z


# revision 26
# speedup vs baseline: 1.0529x; 1.0199x over previous
"""CovariantEvolutionBlock Trainium2 kernel (v2).

Strategy: token-parallel over B*L across 8 cores (512 tokens/core), zero
collectives. Each core recomputes full-batch K/V for attention (inputs are
rotated per-core so "own" tokens are always columns 0:512; sigmoid attention
is permutation-invariant over keys). Activations are kept feature-major
[dims, tokens] on-chip so matmul chains need no transposes.

All dense linears except cu*/m* run in fp8(e4m3) with DoubleRow perf mode;
weights are host-scaled by 256 to escape the e4m3 subnormal range and
descaled inside the PSUM evictions. cu/m stay bf16 (fp8 there costs ~2.4e-2
rel err, over the 2e-2 gate).

v2 scheduling changes vs v1:
- DMAs spread across engine queues (weights/biases on scalar, vw + z32/c32
  prefetch on gpsimd, activations+slabs on sync); big memsets on gpsimd.
- rms square-sums run as fp8 DoubleRow matmuls against a [128,2,1] ones
  stationary (half the passes), with x^2 computed fp8*fp8 on vector.
- g1 interleaved into the attention loop (tensor has slack there: the phase
  is scalar-sigmoid-bound), g2+o right after the head loop; phase4 is
  cu1 -> m1 -> cu2 -> m2 with cu1's contraction reordered (conn, dz first,
  z1 last) so each linear starts before its predecessor's tail lands.
- weight slabs prefetch 2-deep from a pool opened before attention.
- raw z/c staged per-chunk to bf16 (halves their SBUF) -- z1/conn_new pick
  up ~0.3% extra rel err from the bf16 carry, well within budget.
"""

import sys

try:
    import concourse.bass as bass  # noqa: F401
except ImportError:
    sys.path.insert(0, "/opt/trn_rl_repo")

import numpy as np
import ml_dtypes

import concourse.bacc as bacc
import concourse.tile as tile
import concourse.mybir as mybir
from concourse.bass_utils import run_bass_kernel_spmd

F32 = mybir.dt.float32
BF16 = mybir.dt.bfloat16
FP8 = mybir.dt.float8e4
AF = mybir.ActivationFunctionType
ALU = mybir.AluOpType
DR = mybir.MatmulPerfMode.DoubleRow

B, L, D, H, HD = 2, 2048, 1024, 16, 64
EPS = 1e-6
NCORES = 8
TOK = 512          # own tokens per core
KEYS = 2048        # keys per batch
KC = D // 128      # 8 feature chunks of 128
NTB = KEYS // TOK  # 4 token blocks per batch
OBW = 256          # out-block width (2 m-chunks) per psum tile
WSCALE = 256.0     # fp8 weight scale (weights ~N(0,0.02) are subnormal raw)
DS = 1.0 / WSCALE

# name -> (n_in, n_out) for ob-major packed linears
LINS = {
    "fw1": (D, 2 * D), "fw2": (2 * D, D),
    "gw1": (2 * D, D), "gw2": (D, D),
    "qw": (2 * D, D), "kw": (2 * D, D),
    "ow": (D, D),
    "cuw1": (3 * D, 2 * D), "cuw2": (2 * D, D),
    "mw1": (D, 4 * D), "mw2": (4 * D, D),
}
# cu/m weight+act quantization lands unattenuated on the outputs
# (conn_new = c + cu2(...), z2 = z1 + m2(...)): fp8 there measures ~2.4e-2
# end-to-end, over the 2e-2 gate. Everything else is attenuated (dt=0.1 on
# dz, 1/rel_sum on ctx) -> fp8 ok.
FP8_LINS = {"fw1", "fw2", "gw1", "gw2", "qw", "kw", "ow", "cuw1", "mw1"}


def _bias_ap(dram_ap):
    # [dim] -> [128, dim//128]: tile[p, c] = bias[c*128 + p]
    return dram_ap.rearrange("(c p) -> p c", p=128)


def build_program(dt_val: float, temp_val: float):
    nc = bacc.Bacc("TRN2", target_bir_lowering=False, debug=False,
                   num_devices=NCORES)

    d_in = {}
    for name, shape, dt in [
        ("zT", [D, KEYS], F32), ("cT", [D, KEYS], F32),
        ("vwK", [128, KC, D], FP8),  # k-major: streamed operand
        ("fb1", [128, 16], F32), ("fb2", [128, 8], F32),
        ("gb1", [128, 8], F32), ("gb2", [128, 8], F32),
        ("cub1", [128, 16], F32), ("cub2", [128, 8], F32),
        ("mb1", [128, 32], F32), ("mb2", [128, 8], F32),
        ("wz", [128, 8], F32), ("wc", [128, 8], F32),
        ("wmlp", [128, 8], F32),
    ]:
        d_in[name] = nc.dram_tensor(name, shape, dt, kind="ExternalInput").ap()
    for name, (n_in, n_out) in LINS.items():
        wdt = FP8 if name in FP8_LINS else BF16
        d_in[name + "P"] = nc.dram_tensor(
            name + "P", [128, n_out // OBW, n_in // 128, OBW], wdt,
            kind="ExternalInput").ap()

    z2T_d = nc.dram_tensor("z2T", [D, TOK], F32, kind="ExternalOutput").ap()
    connT_d = nc.dram_tensor("connT", [D, TOK], F32, kind="ExternalOutput").ap()

    sig_scale = float(temp_val) * (HD ** -0.5)

    with tile.TileContext(nc) as tc:
        _emit(nc, tc, d_in, z2T_d, connT_d, float(dt_val), sig_scale)
    nc.compile()
    return nc


def _emit(nc, tc, d_in, z2T_d, connT_d, dt_val, sig_scale):
    from contextlib import ExitStack

    ctx = ExitStack()
    with ctx:
        # ---------- persistent pools ----------
        const = ctx.enter_context(tc.tile_pool(name="const", bufs=1))
        persist = ctx.enter_context(tc.tile_pool(name="persist", bufs=1))

        # rms weights + ones/eps needed immediately; weight/bias DMAs ride
        # the scalar queue so sync is free for the z/c activation chunks.
        # The V-projection weight goes first: V's matmuls need it ~12us in.
        vw_pool = ctx.enter_context(tc.tile_pool(name="vwp", bufs=1))
        vw = vw_pool.tile([128, KC, D], FP8, tag="vw")
        nc.scalar.dma_start(out=vw[:], in_=d_in["vwK"][:, :, :])
        wcol = {}
        for name in ["wz", "wc", "wmlp"]:
            t = const.tile([128, KC], F32, tag=name)
            nc.scalar.dma_start(out=t[:], in_=d_in[name][:, :])
            wcol[name] = t
        ones_col = const.tile([128, 1], BF16, tag="ones")
        nc.vector.memset(ones_col[:], 1.0)
        # [128,2,16] so the DR chunk-pair stride is 16B-aligned; used as
        # a [128,2,1] slice
        ones2 = const.tile([128, 2, 16], FP8, tag="ones2")
        nc.vector.memset(ones2[:], 1.0)
        eps1 = const.tile([1, 1], F32, tag="eps1")
        nc.vector.memset(eps1[:], EPS)
        warm = const.tile([1, 2], F32, tag="warm")
        nc.scalar.activation(warm[0:1, 0:1], eps1[:], AF.Square)
        nc.scalar.activation(warm[0:1, 1:2], eps1[:], AF.Sqrt)
        vwarm = const.tile([128, 480], BF16, tag="vwarm")
        nc.vector.memset(vwarm[:], 0.0)
        nc.vector.tensor_copy(vwarm[:], vwarm[:])

        # persistent activations (own tokens, feature-major, fp8)
        cn_own = persist.tile([128, KC, TOK], FP8, tag="cn_own")
        attnT = persist.tile([128, KC, TOK], FP8, tag="attnT")
        mlp = ctx.enter_context(tc.tile_pool(name="mlp", bufs=1))
        mlp2 = ctx.enter_context(tc.tile_pool(name="mlp2", bufs=1))
        outp = ctx.enter_context(tc.tile_pool(name="outp", bufs=1))

        bias = {}

        def load_biases(names):
            for name in names:
                n = d_in[name].shape[1]
                t = const.tile([128, n], F32, tag=name)
                nc.scalar.dma_start(out=t[:], in_=d_in[name][:, :])
                bias[name] = t

        # ---------- generic feature-major linear ----------
        # Weights stream as one packed slab per 256-wide out-block.
        # fp8 linears use DoubleRow (rhs_fn returns [128,2,TOK] pairs);
        # bf16 linears use plain matmul (rhs_fn returns [128,TOK] chunks).
        def _emit_ob(wname, ob, rhs_fn, evict_fn, wpool, pspool, wk,
                     korder=None):
            n_in, n_out = LINS[wname]
            fp8 = wname in FP8_LINS
            wdt = FP8 if fp8 else BF16
            kcn = n_in // 128
            ksteps = kcn // 2 if fp8 else kcn
            ks = list(range(ksteps)) if korder is None else korder
            wP = d_in[wname + "P"]
            w = wpool.tile([128, wk, OBW], wdt, tag="wslab")
            nc.sync.dma_start(out=w[:, 0:kcn, :], in_=wP[:, ob, :, :])
            ps = pspool.tile([128, 2, 512], F32, tag="lin")
            for i, k in enumerate(ks):
                for m in range(2):
                    if fp8:
                        nc.tensor.matmul(
                            ps[:, m, :TOK],
                            w[:, 2 * k:2 * k + 2, m * 128:(m + 1) * 128],
                            rhs_fn(k), start=(i == 0),
                            stop=(i == ksteps - 1), perf_mode=DR)
                    else:
                        nc.tensor.matmul(
                            ps[:, m, :TOK],
                            w[:, k, m * 128:(m + 1) * 128],
                            rhs_fn(k), start=(i == 0),
                            stop=(i == ksteps - 1))
            evict_fn(ob, ps[:, :, :TOK])

        def linear_fm(wname, rhs_fn, evict_fn, wpool, pspool, wk=32,
                      k_order=None):
            for ob in range(LINS[wname][1] // OBW):
                _emit_ob(wname, ob, rhs_fn, evict_fn, wpool, pspool, wk,
                         k_order)

        # ---------- phase 1+2: norms, K, V, Q, f1, f2 ----------
        with tc.tile_pool(name="kvq", bufs=1) as kvq:
            KT = kvq.tile([128, KC, KEYS], BF16, tag="KT")
            V_sb = kvq.tile([128, H, H, HD + 1], FP8, tag="V")
            QT_z = kvq.tile([128, H, TOK], BF16, tag="QT")


            with (
                tc.tile_pool(name="tmp12", bufs=1) as tmp12,
                tc.tile_pool(name="wpool12", bufs=2) as wpool12,
                tc.tile_pool(name="ps_lin", bufs=3, space="PSUM") as ps_lin,
            ):
                zn_own = tmp12.tile([128, KC, TOK], FP8, tag="zn_own")
                fh = tmp12.tile([128, 2 * KC, TOK], FP8, tag="fh")

                dzl_b8 = mlp.tile([128, KC, TOK], FP8, tag="dzl8")

                norm_scope = ExitStack()
                nrm = norm_scope.enter_context(
                    tc.tile_pool(name="nrm", bufs=1))
                xrawp = norm_scope.enter_context(
                    tc.tile_pool(name="xraw", bufs=3))
                ps_ss = norm_scope.enter_context(
                    tc.tile_pool(name="ps_ss", bufs=2, space="PSUM"))

                def evict_silu(dst, ps, bias_ap):
                    # silu(x) = x * sigmoid(x), x = DS*ps + b  (b == 0 here;
                    # the linear term drops it, the sigmoid keeps it).
                    # dst/ps are [128, 2, TOK] out-block pairs.
                    sg = mlp.tile([128, 2, TOK], BF16, tag="sg", bufs=2)
                    nc.scalar.activation(sg[:], ps, AF.Sigmoid, bias=bias_ap,
                                         scale=DS)
                    nc.vector.scalar_tensor_tensor(
                        dst, ps, DS, sg[:], op0=ALU.mult, op1=ALU.mult)

                def norm_block(xT_d, w_t, dst):
                    # normed fp8 chunks into dst [128, KC, TOK].
                    # pair-granular: one DMA per 2 chunks, one copy, one
                    # fp8 square, one DoubleRow ss pass -- short latency
                    # chain, few semaphore hops.
                    ss = ps_ss.tile([1, TOK], F32, tag="ss")
                    for kp in range(KC // 2):
                        sq = xrawp.tile([128, 2, TOK], FP8, tag="sq",
                                        bufs=2)
                        xf = xrawp.tile([128, 2, TOK], F32, tag="xf",
                                        bufs=3)
                        nc.sync.dma_start(out=xf[:], in_=xT_d[kp])
                        dpair = dst[:, 2 * kp:2 * kp + 2, :]
                        nc.vector.tensor_copy(dpair, xf[:])
                        if kp % 2 == 0:
                            nc.scalar.activation(sq[:], dpair, AF.Square)
                        else:
                            nc.vector.tensor_mul(sq[:], dpair, dpair)
                        nc.tensor.matmul(ss[:], ones2[:, :, 0:1], sq[:],
                                         start=(kp == 0),
                                         stop=(kp == KC // 2 - 1),
                                         perf_mode=DR)
                    sf = xrawp.tile([1, TOK], F32, tag="sf", bufs=1)
                    nc.scalar.activation(sf[:], ss[:], AF.Sqrt,
                                         bias=eps1[:], scale=1.0 / D)
                    rcp = xrawp.tile([1, 2, TOK], F32, tag="rcp", bufs=1)
                    nc.vector.reciprocal_approx_accurate(
                        rcp[0:1, 0, :], sf[:], rcp[0:1, 1, :])
                    bc = xrawp.tile([128, TOK], F32, tag="bc", bufs=1)
                    nc.gpsimd.partition_broadcast(bc[:], rcp[0:1, 0, :])
                    for k in range(KC):
                        nc.vector.scalar_tensor_tensor(
                            dst[:, k, :], bc[:], w_t[:, k:k + 1],
                            dst[:, k, :], op0=ALU.mult, op1=ALU.mult)

                for tb in range(NTB):
                    cols = slice(tb * TOK, (tb + 1) * TOK)
                    zslices = [
                        d_in["zT"][2 * kp * 128:(2 * kp + 2) * 128, cols]
                        .rearrange("(two p) t -> p two t", p=128)
                        for kp in range(KC // 2)]
                    cslices = [
                        d_in["cT"][2 * kp * 128:(2 * kp + 2) * 128, cols]
                        .rearrange("(two p) t -> p two t", p=128)
                        for kp in range(KC // 2)]
                    if tb == 0:
                        zn_tb, cn_tb = zn_own, cn_own
                    else:
                        zn_tb = nrm.tile([128, KC, TOK], FP8, tag="zn_tb",
                                         bufs=2)
                        cn_tb = nrm.tile([128, KC, TOK], FP8, tag="cn_tb",
                                         bufs=2)
                    norm_block(zslices, wcol["wz"], zn_tb)

                    if tb == 0:
                        load_biases(["fb1"])
                    elif tb == 2:
                        load_biases(["fb2", "gb1", "gb2",
                                     "cub1", "cub2", "mb1", "mb2"])

                    # V first: needs only zn, overlaps the c-norm
                    for kc4 in range(4):
                        kcg = tb * 4 + kc4
                        ps = ps_lin.tile([128, 2, 512], F32, tag="lin")
                        for k in range(KC // 2):
                            lhs = zn_tb[:, 2 * k:2 * k + 2,
                                        kc4 * 128:(kc4 + 1) * 128]
                            for vb in range(2):
                                nc.tensor.matmul(
                                    ps[:, vb, :], lhs,
                                    vw[:, 2 * k:2 * k + 2,
                                       vb * 512:(vb + 1) * 512],
                                    start=(k == 0),
                                    stop=(k == KC // 2 - 1),
                                    perf_mode=DR)
                        for vb in range(2):
                            src = ps[:, vb, :].rearrange(
                                "p (h d) -> p h d", h=8)
                            nc.scalar.activation(
                                V_sb[:, kcg, vb * 8:(vb + 1) * 8, 0:HD],
                                src, AF.Copy, scale=DS)

                    norm_block(cslices, wcol["wc"], cn_tb)

                    if tb == 0:
                        # gpsimd is free here; QT zeros must beat Q's evicts
                        nc.gpsimd.memset(QT_z[:], 0.0)

                        # f1 needs only zn_own: fills the tensor queue
                        # while the c-norm tail chain completes
                        def f1_evict(ob, ps):
                            evict_silu(fh[:, 2 * ob:2 * ob + 2, :], ps,
                                       bias["fb1"][:, 2 * ob:2 * ob + 1])

                        linear_fm("fw1",
                                  lambda k: zn_own[:, 2 * k:2 * k + 2, :],
                                  f1_evict, wpool12, ps_lin, wk=16)

                    # K for this token block -> KT[:, :, tb]
                    def k_rhs2(k):
                        return (zn_tb[:, 2 * k:2 * k + 2, :] if k < KC // 2
                                else cn_tb[:, 2 * k - KC:2 * k - KC + 2, :])

                    def k_evict(ob, ps):
                        nc.scalar.activation(KT[:, 2 * ob:2 * ob + 2, cols],
                                             ps, AF.Copy, scale=DS)

                    linear_fm("kw", k_rhs2, k_evict, wpool12, ps_lin, wk=16)

                    if tb == 0:
                        # Q projection (own tokens), zero-padded per head
                        def q_rhs2(k):
                            return (zn_own[:, 2 * k:2 * k + 2, :]
                                    if k < KC // 2
                                    else cn_own[:, 2 * k - KC:
                                                2 * k - KC + 2, :])

                        def q_evict(ob, ps):
                            for j in range(2):
                                mc = 2 * ob + j
                                nc.scalar.activation(
                                    QT_z[0:64, 2 * mc, :], ps[0:64, j, :],
                                    AF.Copy, scale=DS)
                                nc.scalar.activation(
                                    QT_z[64:128, 2 * mc + 1, :],
                                    ps[64:128, j, :], AF.Copy, scale=DS)

                        linear_fm("qw", q_rhs2, q_evict, wpool12, ps_lin,
                                  wk=16)

                norm_scope.close()

                nc.gpsimd.memset(V_sb[:, :, :, HD:HD + 1], 1.0)

                # f2: dz_local (fp8; the extra quantization on dz is
                # attenuated by dt=0.1 downstream)
                def f2_evict(ob, ps):
                    nc.vector.tensor_scalar_mul(
                        dzl_b8[:, 2 * ob:2 * ob + 2, :], ps, DS)

                linear_fm("fw2", lambda k: fh[:, 2 * k:2 * k + 2, :],
                          f2_evict, wpool12, ps_lin, wk=16)

                # prefetch raw z/c (own block) during attention on the
                # gpsimd queue, staging fp32 -> bf16 on gpsimd
                z32 = mlp.tile([128, KC, TOK], BF16, tag="z32")
                c32 = mlp.tile([128, KC, TOK], BF16, tag="c32")
                c8 = mlp2.tile([128, KC, TOK], FP8, tag="c8")
                # staging DMAs ride the sync queue: their readiness-at-t0
                # otherwise hoists the whole chain into tb0's gpsimd
                # program, starving the norm broadcasts there. The copies
                # chain behind the DMAs by data deps, so they land late too.
                for k in range(KC):
                    stga = mlp.tile([128, TOK], F32, tag="zcstage", bufs=2)
                    nc.sync.dma_start(
                        out=stga[:],
                        in_=d_in["zT"][k * 128:(k + 1) * 128, 0:TOK])
                    nc.gpsimd.tensor_copy(z32[:, k, :], stga[:])
                    stgb = mlp.tile([128, TOK], F32, tag="zcstage", bufs=2)
                    nc.sync.dma_start(
                        out=stgb[:],
                        in_=d_in["cT"][k * 128:(k + 1) * 128, 0:TOK])
                    nc.gpsimd.tensor_copy(c32[:, k, :], stgb[:])
                    nc.gpsimd.tensor_copy(c8[:, k, :], stgb[:])

            # ---------- phase 3: sigmoid attention + g1/g2/o ----------
            gh = mlp2.tile([128, KC, TOK], FP8, tag="gh")
            s_b = mlp2.tile([128, KC, TOK], BF16, tag="s_b")
            z1_b = mlp2.tile([128, KC, TOK], BF16, tag="z1b")

            with (
                tc.tile_pool(name="rel", bufs=1) as relp,
                tc.tile_pool(name="att_s", bufs=2) as attsp,
                tc.tile_pool(name="wpool_att", bufs=2) as wpool_att,
                tc.tile_pool(name="ps_g", bufs=1, space="PSUM") as ps_g,
            ):
                att_scope = ExitStack()
                ps_sc = att_scope.enter_context(
                    tc.tile_pool(name="ps_sc", bufs=2, space="PSUM"))
                ps_av = att_scope.enter_context(
                    tc.tile_pool(name="ps_av", bufs=2, space="PSUM"))

                # gh = tanh(cat(cn, dzl) @ g_w1.T + gb1), interleaved into
                # the attention loop (tensor slack; phase is sigmoid-bound)
                def g1_evict(ob, ps):
                    nc.scalar.activation(gh[:, 2 * ob:2 * ob + 2, :], ps,
                                         AF.Tanh,
                                         bias=bias["gb1"][:, 2 * ob:2 * ob + 1],
                                         scale=DS)

                def g1_rhs(k):
                    return (cn_own[:, 2 * k:2 * k + 2, :] if k < KC // 2
                            else dzl_b8[:, 2 * k - KC:2 * k - KC + 2, :])

                for h in range(H):
                    rel = relp.tile([128, H, TOK], FP8, tag="rel", bufs=2)
                    for kc2 in range(H // 2):
                        sc = ps_sc.tile([128, 2, TOK], F32, tag="sc")
                        for j in range(2):
                            kc = 2 * kc2 + j
                            nc.tensor.matmul(
                                sc[:, j, :],
                                KT[:, h // 2, kc * 128:(kc + 1) * 128],
                                QT_z[:, h, :], start=True, stop=True)
                        nc.scalar.activation(
                            rel[:, 2 * kc2:2 * kc2 + 2, :], sc[:],
                            AF.Sigmoid, scale=sig_scale)
                    av = ps_av.tile([65, TOK], F32, tag="av")
                    for kc in range(H // 2):
                        nc.tensor.matmul(av[:],
                                         V_sb[:, 2 * kc:2 * kc + 2, h, :],
                                         rel[:, 2 * kc:2 * kc + 2, :],
                                         start=(kc == 0),
                                         stop=(kc == H // 2 - 1),
                                         perf_mode=DR)
                    # attn = av / max(rel_sum, 1)
                    rs = attsp.tile([1, 3, TOK], F32, tag="rs")
                    nc.vector.tensor_scalar_max(rs[0:1, 0, :],
                                                av[64:65, :], 1.0)
                    nc.vector.reciprocal_approx_accurate(
                        rs[0:1, 1, :], rs[0:1, 0, :], rs[0:1, 2, :])
                    bcv = attsp.tile([64, TOK], F32, tag="bcv")
                    nc.gpsimd.partition_broadcast(bcv[:], rs[0:1, 1, :])
                    po = (h % 2) * 64
                    nc.vector.tensor_mul(attnT[po:po + 64, h // 2, :],
                                         av[0:64, :], bcv[:])

                    if h in (3, 6, 9, 12):
                        _emit_ob("gw1", (h - 3) // 3, g1_rhs, g1_evict,
                                 wpool_att, ps_g, 16)

                # heads done: release score/av psum, open a rotation pool
                # for g2 + o (fills the tail while head 15's attnT lands)
                att_scope.close()
                with tc.tile_pool(name="ps_go", bufs=2,
                                  space="PSUM") as ps_go:
                    # s = dzl + gh @ g_w2.T   (dz = dt*s)
                    def g2_evict(ob, ps):
                        nc.vector.scalar_tensor_tensor(
                            s_b[:, 2 * ob:2 * ob + 2, :], ps, DS,
                            dzl_b8[:, 2 * ob:2 * ob + 2, :],
                            op0=ALU.mult, op1=ALU.add)

                    linear_fm("gw2", lambda k: gh[:, 2 * k:2 * k + 2, :],
                              g2_evict, wpool_att, ps_go, wk=16)

                    # ctx = attn @ o_w.T ; z1 = z + dt*s + ctx
                    def o_evict(ob, ps):
                        t = mlp2.tile([128, 2, TOK], F32, tag="t_z1",
                                      bufs=2)
                        nc.vector.scalar_tensor_tensor(
                            t[:], ps, DS, z32[:, 2 * ob:2 * ob + 2, :],
                            op0=ALU.mult, op1=ALU.add)
                        nc.vector.scalar_tensor_tensor(
                            z1_b[:, 2 * ob:2 * ob + 2, :],
                            s_b[:, 2 * ob:2 * ob + 2, :], dt_val, t[:],
                            op0=ALU.mult, op1=ALU.add)

                    linear_fm("ow", lambda k: attnT[:, 2 * k:2 * k + 2, :],
                              o_evict, wpool_att, ps_go, wk=16)

        # ---------- phase 4: z1 norm, cu, final MLP ----------
        with (
            tc.tile_pool(name="mlp4", bufs=1) as mlp4,
            tc.tile_pool(name="wpool4", bufs=2) as wpool4,
            tc.tile_pool(name="ps_lin4", bufs=3, space="PSUM") as ps_lin4,
            tc.tile_pool(name="ps_ss2", bufs=2, space="PSUM") as ps_ss2,
        ):
            # z1n = rms(z1) * wmlp  (frees the norm chain before m1)
            z1n = mlp4.tile([128, KC, TOK], FP8, tag="z1n")
            ss = ps_ss2.tile([1, TOK], F32, tag="ss2")
            for k in range(KC):
                sq = mlp4.tile([128, TOK], BF16, tag="sq2", bufs=2)
                nc.vector.tensor_mul(sq[:], z1_b[:, k, :], z1_b[:, k, :])
                nc.tensor.matmul(ss[:], ones_col[:], sq[:],
                                 start=(k == 0), stop=(k == KC - 1))
            sf = mlp4.tile([1, 3, TOK], F32, tag="sf2")
            nc.scalar.activation(sf[0:1, 0, :], ss[:], AF.Sqrt, bias=eps1[:],
                                 scale=1.0 / D)
            nc.vector.reciprocal_approx_accurate(
                sf[0:1, 1, :], sf[0:1, 0, :], sf[0:1, 2, :])
            bc2 = mlp4.tile([128, TOK], F32, tag="bc2")
            nc.gpsimd.partition_broadcast(bc2[:], sf[0:1, 1, :])
            for k in range(KC):
                nc.vector.scalar_tensor_tensor(
                    z1n[:, k, :], bc2[:], wcol["wmlp"][:, k:k + 1],
                    z1_b[:, k, :], op0=ALU.mult, op1=ALU.mult)

            # cu: du = silu(cat(c, z1, dt*s) @ cu_w1.T + cub1), fp8 DR.
            # fp8 stages of c/z1/s copied on gpsimd (idle here).
            z1_8 = mlp4.tile([128, KC, TOK], FP8, tag="z1_8")
            s8 = mlp4.tile([128, KC, TOK], FP8, tag="s8")
            for k in range(KC):
                nc.gpsimd.tensor_copy(s8[:, k, :], s_b[:, k, :])
                nc.gpsimd.tensor_copy(z1_8[:, k, :], z1_b[:, k, :])
            du = mlp4.tile([128, 32, TOK], BF16, tag="hid")

            def cu1_rhs(kp):
                if kp < KC // 2:
                    return c8[:, 2 * kp:2 * kp + 2, :]
                if kp < KC:
                    return z1_8[:, 2 * kp - KC:2 * kp - KC + 2, :]
                return s8[:, 2 * kp - 2 * KC:2 * kp - 2 * KC + 2, :]

            def cu1_evict(ob, ps):
                evict_silu(du[:, 2 * ob:2 * ob + 2, :], ps,
                           bias["cub1"][:, 2 * ob:2 * ob + 1])

            # pair order: conn, dz (ready early), z1 last
            cu1_korder = (list(range(0, KC // 2))
                          + list(range(KC, 3 * KC // 2))
                          + list(range(KC // 2, KC)))
            linear_fm("cuw1", cu1_rhs, cu1_evict, wpool4, ps_lin4,
                      k_order=cu1_korder)

            # mh = silu(z1n @ m_w1.T + mb1), fp8 DR
            mh = mlp4.tile([128, 32, TOK], BF16, tag="hid2")

            def m1_evict(ob, ps):
                evict_silu(mh[:, 2 * ob:2 * ob + 2, :], ps,
                           bias["mb1"][:, 2 * ob:2 * ob + 1])

            linear_fm("mw1", lambda k: z1n[:, 2 * k:2 * k + 2, :],
                      m1_evict, wpool4, ps_lin4)

            # conn_new = c + du @ cu_w2.T  (after m1: du long since ready,
            # and m2's wait on mh's tail hides under cu2)
            def cu2_evict(ob, ps):
                co = outp.tile([128, 2, TOK], F32, tag="co")
                nc.vector.scalar_tensor_tensor(
                    co[:], ps, bias["cub2"][:, 2 * ob:2 * ob + 1],
                    c32[:, 2 * ob:2 * ob + 2, :],
                    op0=ALU.add, op1=ALU.add)
                nc.sync.dma_start(
                    out=connT_d[ob * 256:(ob + 1) * 256, :]
                    .rearrange("(two p) t -> p two t", p=128), in_=co[:])

            linear_fm("cuw2", lambda k: du[:, k, :],
                      cu2_evict, wpool4, ps_lin4)

            # z2 = z1 + mh @ m_w2.T
            def m2_evict(ob, ps):
                zo = outp.tile([128, 2, TOK], F32, tag="zo")
                nc.vector.scalar_tensor_tensor(
                    zo[:], ps, bias["mb2"][:, 2 * ob:2 * ob + 1],
                    z1_b[:, 2 * ob:2 * ob + 2, :],
                    op0=ALU.add, op1=ALU.add)
                nc.sync.dma_start(
                    out=z2T_d[ob * 256:(ob + 1) * 256, :]
                    .rearrange("(two p) t -> p two t", p=128), in_=zo[:])

            linear_fm("mw2", lambda k: mh[:, k, :],
                      m2_evict, wpool4, ps_lin4)


_CACHE = {}


def _pack_ob(wT, fp8):
    # wT [n_in, n_out] -> [128, nob, kcn, OBW]:
    # packed[p, ob, k, m] = wT[k*128+p, ob*OBW+m]  (*WSCALE if fp8)
    n_in, n_out = wT.shape
    kcn, nob = n_in // 128, n_out // OBW
    if fp8:
        wT = wT * WSCALE
    dt = ml_dtypes.float8_e4m3 if fp8 else ml_dtypes.bfloat16
    return np.ascontiguousarray(
        wT.reshape(kcn, 128, nob, OBW).transpose(1, 2, 0, 3)).astype(dt)


def _prep_shared(inputs):
    def t(x):
        return np.ascontiguousarray(np.asarray(x, np.float32).T)

    dt_val = float(np.asarray(inputs["dt"]))
    cu1 = np.asarray(inputs["cu_w1"], np.float32).copy()
    cu1[:, 2 * D:] *= dt_val  # fold dz = dt*s into cu_w1's dz block
    wT = {
        "fw1": t(inputs["f_w1"]), "fw2": t(inputs["f_w2"]),
        "gw1": t(inputs["g_w1"]), "gw2": t(inputs["g_w2"]),
        "qw": t(inputs["q_w"]), "kw": t(inputs["k_w"]),
        "ow": t(inputs["o_w"]),
        "cuw1": np.ascontiguousarray(cu1.T),
        "cuw2": t(inputs["cu_w2"]),
        "mw1": t(inputs["m_w1"]), "mw2": t(inputs["m_w2"]),
    }
    shared = {name + "P": _pack_ob(w, name in FP8_LINS)
              for name, w in wT.items()}
    # vw: k-major [128, kcn, n_out] (streamed as moving operand)
    vwT = t(inputs["v_w"])
    shared["vwK"] = np.ascontiguousarray(
        (vwT * WSCALE).reshape(KC, 128, D).transpose(1, 0, 2)
    ).astype(ml_dtypes.float8_e4m3)
    for name, key in [("fb1", "f_b1"), ("fb2", "f_b2"), ("gb1", "g_b1"),
                      ("gb2", "g_b2"), ("cub1", "cu_b1"), ("cub2", "cu_b2"),
                      ("mb1", "m_b1"), ("mb2", "m_b2"), ("wz", "w_z"),
                      ("wc", "w_c"), ("wmlp", "w_mlp")]:
        # [n] -> [128, n//128] with tile[p, c] = b[c*128 + p]
        arr = np.asarray(inputs[key], np.float32)
        shared[name] = np.ascontiguousarray(arr.reshape(-1, 128).T)
    return shared


def kernel(**inputs):
    z = np.asarray(inputs["z"], np.float32)
    conn = np.asarray(inputs["connection"], np.float32)
    dt_val = float(np.asarray(inputs["dt"]))
    temp_val = float(np.asarray(inputs["temp"]))

    key = (dt_val, temp_val)
    if key not in _CACHE:
        _CACHE[key] = build_program(dt_val, temp_val)
    nc = _CACHE[key]

    shared = _prep_shared(inputs)
    zT = [np.ascontiguousarray(z[b].T) for b in range(B)]
    cT = [np.ascontiguousarray(conn[b].T) for b in range(B)]

    in_maps = []
    for c in range(NCORES):
        b, tb = divmod(c, NTB)
        m = dict(shared)
        m["zT"] = np.ascontiguousarray(np.roll(zT[b], -tb * TOK, axis=1))
        m["cT"] = np.ascontiguousarray(np.roll(cT[b], -tb * TOK, axis=1))
        in_maps.append(m)

    res = run_bass_kernel_spmd(nc, in_maps, list(range(NCORES)))

    z2 = np.empty((B, L, D), np.float32)
    conn_new = np.empty((B, L, D), np.float32)
    for c in range(NCORES):
        b, tb = divmod(c, NTB)
        sl = slice(tb * TOK, (tb + 1) * TOK)
        z2[b, sl, :] = res.results[c]["z2T"].T
        conn_new[b, sl, :] = res.results[c]["connT"].T
    return z2, conn_new, z
